# revision 8
# baseline (speedup 1.0000x reference)
"""Location-dependent 3D conv (AsymConv) on 8 TRN2 NeuronCores.

Math (per output voxel):
    out[b, 0, x, y, z] = sum_{i,j,l in 0..2} Xp[b, x+i, y+j, z+l] * W[x, y, z, (i*3+j)*3+l]
with Xp = edge-padded X by 1 plane on each spatial side.

Strategy (v6; evolved from the 55.9 us v2 via trace analysis):
  - Shard the X spatial axis (96 = 8 cores x 12 planes); host ships halo'd
    shards -> no inter-core communication.
  - Per-core SBUF layout: partition dim = y (96 of 128); free = (b, x, z).
    y-shifts come from 3 j-copies: j=1 host-shipped, j=0/j=2 built on-chip
    by partition-shifted SBUF->SBUF DMAs (no HBM cost, no PE/ACT cost);
    x/z shifts are free-dim offsets.
  - Products X*W run on the DVE in fp16 2x mode; that stream is the hard
    floor (~34 us busy: 62k free-dim elems/partition at 2 elem/cycle,
    0.96 GHz). Everything else is scheduled to keep the streak stall-free:
      * the two HWDGE rings share ~280 B/ns of HBM bandwidth, so W units
        ALTERNATE between the rings in global consumption order - at any
        moment the rings carry the next two units needed;
      * each dma_start costs ~0.6-0.8 us of ISSUE time on its ring engine,
        so the stream uses few, large transfers;
      * l=1 taps run as i-triples (one DVE op per (j, b) instead of
        pair+single) via an overlapping x-window AP;
      * the last unit (j=2 triple) is split per psum x-chunk so the six
        PSUM accumulations retire staggered; each (b, chunk) is evacuated
        (ACT copy to fp16) and DMA'd out as soon as its 27th tap lands.
  - The 27-term accumulation runs on the TensorEngine as identity matmuls
    into PSUM fp32 (x-chunks 5/5/2 planes), trailing the DVE by <1 op.
"""

import os

import numpy as np

# ---- problem constants (hardcoded per harness rules) ----
B = 2
D = 96  # Dx = Dy = Dz
KSZ = 3
NTAP = KSZ**3  # 27
NCORES = 8
XS = D // NCORES  # 12 x-planes per core
XH = XS + 2  # with halo
ZP = D + 2  # padded z

F16 = np.float16
LAST_RESULT = None  # BassKernelResults of the most recent run (for test.py)

_GRAPH_CACHE = {}

# psum x-plane chunks of the 12-plane streams (480/480/192 columns)
CHUNKS = [(0, 5), (5, 5), (10, 2)]

# ---- W unit schedule (DMA consumption order) ----
# kinds: "z" = (l0, l2) z-pair for (i, j) [2 taps], "t" = i-triple for l=1, j
# [3 taps: (0,j,1), (1,j,1), (2,j,1)].  j=1 units lead: only the j=1 X copy
# is host-shipped; j=0 / j=2 are built on-chip while j=1 computes.
UNITS = [
    ("z", (0, 1)),
    ("z", (1, 1)),
    ("z", (2, 1)),
    ("z", (0, 0)),
    ("z", (1, 0)),
    ("z", (2, 0)),
    ("z", (0, 2)),
    ("z", (1, 2)),
    ("z", (2, 2)),
    ("t", 1),
    ("t", 0),
    ("t", 2),
]


def _unit_taps(kind, arg):
    if kind == "z":
        i, j = arg
        return [(i, j, 0), (i, j, 2)]
    j = arg
    return [(0, j, 1), (1, j, 1), (2, j, 1)]


def _build_graph():
    """Build (and cache) the per-core Bass graph. Same graph for all 8 cores."""
    if "nc" in _GRAPH_CACHE:
        return _GRAPH_CACHE["nc"]

    from concourse import bacc
    from concourse import bass as _bass
    import concourse.mybir as mybir
    from concourse.tile import TileContext

    f16 = mybir.dt.float16
    f32 = mybir.dt.float32

    nc = bacc.Bacc("TRN2", target_bir_lowering=False, debug=False, num_devices=NCORES)

    # only the j=1 X copy is shipped: x[y', b, x, z] = Xp[y'+1, b, x, z];
    # j=0 / j=2 copies are derived on-chip by partition-shift matmuls
    x_d = nc.dram_tensor("x", [D, B, XH, ZP], f16, kind="ExternalInput")
    # W per unit (y-major), all units concatenated along the free dim
    unit_w = [len(_unit_taps(k, a)) * XS * D for (k, a) in UNITS]
    unit_off = np.concatenate([[0], np.cumsum(unit_w)]).tolist()
    w_d = nc.dram_tensor("w", [D, unit_off[-1]], f16, kind="ExternalInput")
    id_d = nc.dram_tensor("ident", [D, D], f16, kind="ExternalInput")
    out_d = nc.dram_tensor("out", [D, B, XS, D], f16, kind="ExternalOutput")

    with TileContext(nc) as tc:
        with (
            tc.tile_pool(name="xp", bufs=1) as xpool,
            tc.tile_pool(name="wp", bufs=1) as wpool,
            tc.tile_pool(name="pp", bufs=6) as ppool,
            tc.tile_pool(name="psp", bufs=1, space="PSUM") as pspool,
        ):
            # ---- static tiles ----
            x_ts = [
                xpool.tile([D, B, XH, ZP], f16, name=f"x_{j}", tag=f"x_{j}")
                for j in range(KSZ)
            ]
            # z-shifted copies for l == 1 (keeps DVE 2x alignment)
            xz_ts = [
                xpool.tile([D, B, XH, ZP], f16, name=f"xz_{j}", tag=f"xz_{j}")
                for j in range(KSZ)
            ]
            w_ts = []
            for ui, (kind, arg) in enumerate(UNITS):
                nt = len(_unit_taps(kind, arg))
                w_ts.append(
                    wpool.tile(
                        [D, nt, XS, D],
                        f16,
                        name=f"w_{ui}",
                        tag="wz" if nt == 2 else "wt",
                        bufs=9 if nt == 2 else 3,
                    )
                )
            id_t = xpool.tile([D, D], f16, name="id_t", tag="id_t")

            # ---- DMA schedule (HWDGE, both rings, consumption order).
            # Few BIG transfers: each dma_start costs ~0.6-0.8 us of issue
            # time on its ring engine, and completion sems lag the data by
            # ~1 us, so fine-grained pieces only starve the stream. ----
            def w_dma(q, ui, half=None):
                o0, o1 = unit_off[ui], unit_off[ui + 1]
                if half is None:
                    q.dma_start(out=w_ts[ui][:], in_=w_d.ap()[:, o0:o1])
                else:
                    h = (o1 - o0) // 2
                    q.dma_start(
                        out=w_ts[ui][:, half : half + 1],
                        in_=w_d.ap()[:, o0 + half * h : o0 + (half + 1) * h],
                    )

            # ACT ring: ident (gates the first accumulate), the first W unit
            # in halves (the first half gates the very first product), then
            # its share of the alternating unit stream. The j=2 X copy is a
            # partition-shifted SBUF->SBUF dma (replicate padding: edge row
            # duplicated via a 1-row dma), slotted after unit 2.
            nc.scalar.dma_start(out=id_t[:], in_=id_d.ap())
            w_dma(nc.scalar, 0, half=0)
            w_dma(nc.scalar, 0, half=1)
            w_dma(nc.scalar, 2)
            nc.scalar.dma_start(out=x_ts[2][0:95], in_=x_ts[1][1:96])
            nc.scalar.dma_start(out=x_ts[2][95:96], in_=x_ts[1][95:96])
            for ui in (4, 6, 8, 10):
                w_dma(nc.scalar, ui)
            # SP ring: the j=1 X slices (b-split for earliest first product),
            # unit 1, the j=0 X copy (same sbuf->sbuf trick), then its share
            # of the alternating unit stream.
            nc.sync.dma_start(out=x_ts[1][:, 0:1], in_=x_d.ap()[:, 0:1])
            nc.sync.dma_start(out=x_ts[1][:, 1:2], in_=x_d.ap()[:, 1:2])
            w_dma(nc.sync, 1)
            nc.sync.dma_start(out=x_ts[0][1:96], in_=x_ts[1][0:95])
            nc.sync.dma_start(out=x_ts[0][0:1], in_=x_ts[1][0:1])
            for ui in (3, 5, 7, 9, 11):
                w_dma(nc.sync, ui)

            # ---- ScalarE: z-shifted copies (for l == 1 units); j order
            # (1, 0, 2) matches unit consumption order ----
            for j in (1, 0, 2):
                nc.scalar.copy(
                    out=xz_ts[j][:, :, :, 0 : ZP - 1], in_=x_ts[j][:, :, :, 1:ZP]
                )

            # ---- product + accumulate schedule ----
            psums = {
                (b, ci): pspool.tile(
                    [D, nx, D],
                    f32,
                    name=f"ps_{b}_{ci}",
                    tag="ps5" if nx == 5 else "ps2",
                    bufs=4 if nx == 5 else 2,
                )
                for b in range(B)
                for ci, (x0, nx) in enumerate(CHUNKS)
            }

            def zpair_ap(j, b, i):
                """[D, 2, XS, D] view of x_ts[j]: overlapping z-windows l=0,2."""
                base = x_ts[j][:, b, i : i + XS, 0:D]
                ap = list(base.ap)
                return _bass.AP(
                    base.tensor, base.offset, [ap[0], [2, 2], ap[1], ap[2]]
                )

            def xtriple_ap(j, b, x0=0, nx=XS):
                """[D, 3, nx, D] view of xz_ts[j]: overlapping x-windows
                i=0,1,2 (all l=1 taps of one j in a single op)."""
                base = xz_ts[j][:, b, x0 : x0 + nx, 0:D]
                ap = list(base.ap)
                return _bass.AP(
                    base.tensor, base.offset, [ap[0], [ap[1][0], 3], ap[1], ap[2]]
                )

            # per (b, chunk) accumulation counters for start/stop flags
            seen = {(b, ci): 0 for b in range(B) for ci in range(len(CHUNKS))}
            evacuated = set()

            def mm(prod_slice, b, ci):
                s = seen[(b, ci)]
                nc.tensor.matmul(
                    psums[(b, ci)][:],
                    id_t[:],
                    prod_slice,
                    start=(s == 0),
                    stop=(s == NTAP - 1),
                )
                seen[(b, ci)] = s + 1

            def evac(b, ci):
                """PSUM -> SBUF f16 (ACT) -> DRAM, fired as soon as the 27th
                tap of this (b, chunk) has been accumulated."""
                if (b, ci) in evacuated or seen[(b, ci)] != NTAP:
                    return
                evacuated.add((b, ci))
                x0, nx = CHUNKS[ci]
                outsb = ppool.tile(
                    [D, 5, D], f16, name="outsb", tag="outsb", bufs=6
                )[:, 0:nx, :]
                nc.scalar.copy(out=outsb[:], in_=psums[(b, ci)][:])
                q = nc.sync if (b * 3 + ci) % 2 == 0 else nc.scalar
                q.dma_start(out=out_d.ap()[:, b, x0 : x0 + nx, :], in_=outsb[:])

            def consume(prod, b, nt):
                """PE: accumulate nt tap-streams of a product tile into psums.
                Chunk-inner order: consecutive matmuls hit different PSUM banks
                (same-bank back-to-back stalls the accumulate pipeline)."""
                for t in range(nt):
                    for ci, (c0, cn) in enumerate(CHUNKS):
                        mm(prod[:, t, c0 : c0 + cn, :], b, ci)
                for ci in range(len(CHUNKS)):
                    evac(b, ci)

            # -- unit 0 (z-pair (0,1)): two b0 singles (the first waits only
            # on half of the first W tile), then a regular b1 z-pair op --
            i0, j0_ = UNITS[0][1]
            for s, l in enumerate((0, 2)):
                prod = ppool.tile([D, XS, D], f16, name="prod1", tag="prod1", bufs=2)
                nc.vector.tensor_mul(
                    out=prod[:],
                    in0=x_ts[j0_][:, 0, i0 : i0 + XS, l : l + D],
                    in1=w_ts[0][:, s],
                )
                for ci, (c0, cn) in enumerate(CHUNKS):
                    mm(prod[:, c0 : c0 + cn, :], 0, ci)
            prod = ppool.tile([D, 2, XS, D], f16, name="prod2", tag="prod2", bufs=5)
            nc.vector.tensor_mul(out=prod[:], in0=zpair_ap(j0_, 1, i0), in1=w_ts[0][:])
            consume(prod, 1, 2)

            # -- remaining z-pair units --
            for ui in range(1, 9):
                i, j = UNITS[ui][1]
                for b in range(B):
                    prod = ppool.tile(
                        [D, 2, XS, D], f16, name="prod2", tag="prod2", bufs=5
                    )
                    nc.vector.tensor_mul(
                        out=prod[:], in0=zpair_ap(j, b, i), in1=w_ts[ui][:]
                    )
                    consume(prod, b, 2)

            # -- l=1 i-triples; the last unit (j=2) is split per psum chunk
            # so the six accumulations retire staggered --
            for ui in (9, 10):
                j = UNITS[ui][1]
                for b in range(B):
                    prod = ppool.tile(
                        [D, 3, XS, D], f16, name="prod3", tag="prod3", bufs=3
                    )
                    nc.vector.tensor_mul(
                        out=prod[:], in0=xtriple_ap(j, b), in1=w_ts[ui][:]
                    )
                    consume(prod, b, 3)
            j = UNITS[11][1]
            for b in range(B):
                for ci, (x0, nx) in enumerate(CHUNKS):
                    prod = ppool.tile(
                        [D, 3, 5, D], f16, name="prod3c", tag="prod3c", bufs=3
                    )
                    pv = prod[:, :, 0:nx, :] if nx != 5 else prod[:]
                    nc.vector.tensor_mul(
                        out=pv,
                        in0=xtriple_ap(j, b, x0=x0, nx=nx),
                        in1=w_ts[11][:, :, x0 : x0 + nx],
                    )
                    for t in range(3):
                        mm(pv[:, t], b, ci)
                    evac(b, ci)

    nc.compile()
    _GRAPH_CACHE["nc"] = nc
    return nc


def make_in_maps(X, W):
    """Host-side shard prep. X [2,1,96,96,96] f32, W [1,1,96,96,96,27] f32."""
    X = np.asarray(X)
    W = np.asarray(W)
    Xs = X.reshape(B, D, D, D)
    # edge padding on all three spatial dims
    Xp = np.pad(Xs, ((0, 0), (1, 1), (1, 1), (1, 1)), mode="edge")
    # -> [y, b, x, z]
    Xt = np.ascontiguousarray(np.transpose(Xp, (2, 0, 1, 3))).astype(F16)
    W00 = W.reshape(D, D, D, NTAP)  # [x, y, z, tap]
    ident = np.eye(D).astype(F16)

    in_maps = []
    for m in range(NCORES):
        xs_full = Xt[:, :, m * XS : m * XS + XH, :]  # [98, 2, 14, 98]
        im = {"ident": ident}
        # only the j=1 copy: x[y, b, x, z] = Xp[y+1, b, x, z]
        im["x"] = np.ascontiguousarray(xs_full[1 : 1 + D])  # [96, 2, 14, 98]
        wm = W00[m * XS : (m + 1) * XS]  # [12, 96, 96, 27]
        wmt = np.transpose(wm, (1, 0, 2, 3))  # [y, x, z, tap]
        blocks = []
        for kind, arg in UNITS:
            taps = _unit_taps(kind, arg)
            idxs = [(i * KSZ + j) * KSZ + l for (i, j, l) in taps]
            blk = wmt[:, :, :, idxs]  # [y, x, z, nt]
            wt = np.transpose(blk, (0, 3, 1, 2))  # [y, nt, x, z]
            blocks.append(wt.reshape(D, -1))
        im["w"] = np.ascontiguousarray(np.concatenate(blocks, axis=1)).astype(F16)
        in_maps.append(im)
    return in_maps


def kernel(X, W):
    global LAST_RESULT
    from concourse.bass_utils import run_bass_kernel_spmd

    nc = _build_graph()
    in_maps = make_in_maps(X, W)
    trace = bool(int(os.environ.get("ASYM_TRACE", "0")))
    res = run_bass_kernel_spmd(
        nc, in_maps, core_ids=list(range(NCORES)), trace=trace
    )
    LAST_RESULT = res

    out = np.empty((B, 1, D, D, D), dtype=np.float32)
    for m in range(NCORES):
        r = res.results[m]["out"].astype(np.float32)  # [y, b, x, z]
        out[:, 0, m * XS : (m + 1) * XS, :, :] = np.transpose(r, (1, 2, 0, 3))
    return out


# revision 11
# speedup vs baseline: 1.0984x; 1.0984x over previous
"""Location-dependent 3D conv (AsymConv) on 8 TRN2 NeuronCores.

Math (per output voxel):
    out[b, 0, x, y, z] = sum_{i,j,l in 0..2} Xp[b, x+i, y+j, z+l] * W[x, y, z, (i*3+j)*3+l]
with Xp = edge-padded X by 1 plane on each spatial side.

Strategy (v6; evolved from the 55.9 us v2 via trace analysis):
  - Shard the X spatial axis (96 = 8 cores x 12 planes); host ships halo'd
    shards -> no inter-core communication.
  - Per-core SBUF layout: partition dim = y (96 of 128); free = (b, x, z).
    y-shifts come from 3 j-copies (j=1 host-shipped; j=0/j=2 built on-chip
    by PE shifted-identity matmuls); x/z shifts are free-dim offsets.
  - Units are consumed J-GROUPED (all j=1 work, then j=0, then j=2, each
    j's l=1 triple right after its z-pairs) so the on-chip j-copy builds
    and z-shift copies get generous deadlines; the early product ops are
    b-interleaved so the X b1 slice is not needed until ~4 us into the
    streak.
  - Products X*W run on the DVE in fp16 2x mode; that stream is the hard
    floor (~34 us busy: 62k free-dim elems/partition at 2 elem/cycle,
    0.96 GHz). Everything else is scheduled to keep the streak stall-free:
      * the two HWDGE rings share ~280 B/ns of HBM bandwidth, so W units
        ALTERNATE between the rings in global consumption order - at any
        moment the rings carry the next two units needed;
      * each dma_start costs ~0.6-0.8 us of ISSUE time on its ring engine,
        so the stream uses few, large transfers;
      * l=1 taps run as i-triples (one DVE op per (j, b) instead of
        pair+single) via an overlapping x-window AP;
      * the last unit (j=2 triple) is split per psum x-chunk so the six
        PSUM accumulations retire staggered; each (b, chunk) is evacuated
        (ACT copy to fp16) and DMA'd out as soon as its 27th tap lands.
  - The 27-term accumulation runs on the TensorEngine as identity matmuls
    into PSUM fp32 (x-chunks 5/5/2 planes), trailing the DVE by <1 op.
"""

import os

import numpy as np

# ---- problem constants (hardcoded per harness rules) ----
B = 2
D = 96  # Dx = Dy = Dz
KSZ = 3
NTAP = KSZ**3  # 27
NCORES = 8
XS = D // NCORES  # 12 x-planes per core
XH = XS + 2  # with halo
ZP = D + 2  # padded z

F16 = np.float16
LAST_RESULT = None  # BassKernelResults of the most recent run (for test.py)

_GRAPH_CACHE = {}

# psum x-plane chunks of the 12-plane streams (480/480/192 columns)
CHUNKS = [(0, 5), (5, 5), (10, 2)]

# ---- W unit schedule (DMA consumption order) ----
# kinds: "z" = (l0, l2) z-pair for (i, j) [2 taps], "t" = i-triple for l=1, j
# [3 taps: (0,j,1), (1,j,1), (2,j,1)].  j=1 units lead: only the j=1 X copy
# is host-shipped; j=0 / j=2 are built on-chip while j=1 computes.
UNITS = [
    ("z", (0, 1)),
    ("z", (1, 1)),
    ("z", (2, 1)),
    ("t", 1),
    ("z", (0, 0)),
    ("z", (1, 0)),
    ("z", (2, 0)),
    ("t", 0),
    ("z", (0, 2)),
    ("z", (1, 2)),
    ("z", (2, 2)),
    ("t", 2),
]


def _unit_taps(kind, arg):
    if kind == "z":
        i, j = arg
        return [(i, j, 0), (i, j, 2)]
    j = arg
    return [(0, j, 1), (1, j, 1), (2, j, 1)]


def _build_graph():
    """Build (and cache) the per-core Bass graph. Same graph for all 8 cores."""
    if "nc" in _GRAPH_CACHE:
        return _GRAPH_CACHE["nc"]

    from concourse import bacc
    from concourse import bass as _bass
    import concourse.mybir as mybir
    from concourse.tile import TileContext

    f16 = mybir.dt.float16
    f32 = mybir.dt.float32

    nc = bacc.Bacc("TRN2", target_bir_lowering=False, debug=False, num_devices=NCORES)

    # only the j=1 X copy is shipped: x[y', b, x, z] = Xp[y'+1, b, x, z];
    # j=0 / j=2 copies are derived on-chip by partition-shift matmuls
    x_d = nc.dram_tensor("x", [D, B, XH, ZP], f16, kind="ExternalInput")
    # W per unit (y-major), all units concatenated along the free dim
    unit_w = [len(_unit_taps(k, a)) * XS * D for (k, a) in UNITS]
    unit_off = np.concatenate([[0], np.cumsum(unit_w)]).tolist()
    w_d = nc.dram_tensor("w", [D, unit_off[-1]], f16, kind="ExternalInput")
    id_d = nc.dram_tensor("ident", [D, D], f16, kind="ExternalInput")
    sh_d = nc.dram_tensor("shmat", [D, 2, D], f16, kind="ExternalInput")
    out_d = nc.dram_tensor("out", [D, B, XS, D], f16, kind="ExternalOutput")

    with TileContext(nc) as tc:
        with (
            tc.tile_pool(name="xp", bufs=1) as xpool,
            tc.tile_pool(name="wp", bufs=1) as wpool,
            tc.tile_pool(name="pp", bufs=6) as ppool,
            tc.tile_pool(name="psp", bufs=1, space="PSUM") as pspool,
        ):
            # ---- static tiles ----
            x_ts = [
                xpool.tile([D, B, XH, ZP], f16, name=f"x_{j}", tag=f"x_{j}")
                for j in range(KSZ)
            ]
            # z-shifted copies for l == 1 (keeps DVE 2x alignment)
            xz_ts = [
                xpool.tile([D, B, XH, ZP], f16, name=f"xz_{j}", tag=f"xz_{j}")
                for j in range(KSZ)
            ]
            w_ts = []
            for ui, (kind, arg) in enumerate(UNITS):
                nt = len(_unit_taps(kind, arg))
                w_ts.append(
                    wpool.tile(
                        [D, nt, XS, D],
                        f16,
                        name=f"w_{ui}",
                        tag="wz" if nt == 2 else "wt",
                        bufs=9 if nt == 2 else 3,
                    )
                )
            id_t = xpool.tile([D, D], f16, name="id_t", tag="id_t")
            sh_t = xpool.tile([D, 2, D], f16, name="sh_t", tag="sh_t")

            # ---- DMA schedule (HWDGE, both rings, consumption order).
            # Few BIG transfers: each dma_start costs ~0.6-0.8 us of issue
            # time on its ring engine, and completion sems lag the data by
            # ~1 us, so fine-grained pieces only starve the stream. ----
            def w_dma(q, ui, half=None):
                o0, o1 = unit_off[ui], unit_off[ui + 1]
                if half is None:
                    q.dma_start(out=w_ts[ui][:], in_=w_d.ap()[:, o0:o1])
                else:
                    h = (o1 - o0) // 2
                    q.dma_start(
                        out=w_ts[ui][:, half : half + 1],
                        in_=w_d.ap()[:, o0 + half * h : o0 + (half + 1) * h],
                    )

            # ACT ring: ident (gates the first accumulate), the first W unit
            # in halves (the first half gates the very first product), the
            # shift matrices (gate the j-copy builds), then this ring's share
            # of the unit stream (units alternate rings in consumption order
            # so the two rings always carry the next two units needed).
            nc.scalar.dma_start(out=id_t[:], in_=id_d.ap())
            w_dma(nc.scalar, 0, half=0)
            w_dma(nc.scalar, 0, half=1)
            nc.scalar.dma_start(out=sh_t[:], in_=sh_d.ap())
            for ui in (2, 3, 5, 7, 8, 10):
                w_dma(nc.scalar, ui)
            # SP ring: the j=1 X slices (b-split; b1 is not consumed until
            # ~4 us into the streak) + this ring's share of the unit stream.
            nc.sync.dma_start(out=x_ts[1][:, 0:1], in_=x_d.ap()[:, 0:1])
            w_dma(nc.sync, 1)
            nc.sync.dma_start(out=x_ts[1][:, 1:2], in_=x_d.ap()[:, 1:2])
            for ui in (4, 6, 9, 11):
                w_dma(nc.sync, ui)

            # ---- PE: build the j=0 / j=2 X copies from j=1 by partition-
            # shift matmuls; ScalarE evacuates PSUM back to SBUF f16. ACT
            # program order: xz_1 copy first (earliest deadline), then j=0
            # evacs, xz_0, j=2 evacs, xz_2 - each well before its consumer
            # thanks to the j-grouped unit order. ----
            XCH = [(0, 5), (5, 5), (10, 4)]  # XH=14 rows -> <=512 f32 cols
            nc.scalar.copy(
                out=xz_ts[1][:, :, :, 0 : ZP - 1], in_=x_ts[1][:, :, :, 1:ZP]
            )
            for jt, sh_idx in ((0, 0), (2, 1)):
                for b in range(B):
                    for r0, nr in XCH:
                        ps_x = pspool.tile(
                            [D, nr, ZP], f32, name="ps_x", tag="ps_x", bufs=2
                        )
                        nc.tensor.matmul(
                            ps_x[:],
                            sh_t[:, sh_idx, :],
                            x_ts[1][:, b, r0 : r0 + nr, :],
                            start=True,
                            stop=True,
                        )
                        nc.scalar.copy(
                            out=x_ts[jt][:, b, r0 : r0 + nr, :], in_=ps_x[:]
                        )
                nc.scalar.copy(
                    out=xz_ts[jt][:, :, :, 0 : ZP - 1],
                    in_=x_ts[jt][:, :, :, 1:ZP],
                )

            # ---- product + accumulate schedule ----
            psums = {
                (b, ci): pspool.tile(
                    [D, nx, D],
                    f32,
                    name=f"ps_{b}_{ci}",
                    tag="ps5" if nx == 5 else "ps2",
                    bufs=4 if nx == 5 else 2,
                )
                for b in range(B)
                for ci, (x0, nx) in enumerate(CHUNKS)
            }

            def zpair_ap(j, b, i):
                """[D, 2, XS, D] view of x_ts[j]: overlapping z-windows l=0,2."""
                base = x_ts[j][:, b, i : i + XS, 0:D]
                ap = list(base.ap)
                return _bass.AP(
                    base.tensor, base.offset, [ap[0], [2, 2], ap[1], ap[2]]
                )

            def xtriple_ap(j, b, x0=0, nx=XS):
                """[D, 3, nx, D] view of xz_ts[j]: overlapping x-windows
                i=0,1,2 (all l=1 taps of one j in a single op)."""
                base = xz_ts[j][:, b, x0 : x0 + nx, 0:D]
                ap = list(base.ap)
                return _bass.AP(
                    base.tensor, base.offset, [ap[0], [ap[1][0], 3], ap[1], ap[2]]
                )

            # per (b, chunk) accumulation counters for start/stop flags
            seen = {(b, ci): 0 for b in range(B) for ci in range(len(CHUNKS))}
            evacuated = set()

            def mm(prod_slice, b, ci):
                s = seen[(b, ci)]
                nc.tensor.matmul(
                    psums[(b, ci)][:],
                    id_t[:],
                    prod_slice,
                    start=(s == 0),
                    stop=(s == NTAP - 1),
                )
                seen[(b, ci)] = s + 1

            def evac(b, ci):
                """PSUM -> SBUF f16 (ACT) -> DRAM, fired as soon as the 27th
                tap of this (b, chunk) has been accumulated."""
                if (b, ci) in evacuated or seen[(b, ci)] != NTAP:
                    return
                evacuated.add((b, ci))
                x0, nx = CHUNKS[ci]
                outsb = ppool.tile(
                    [D, 5, D], f16, name="outsb", tag="outsb", bufs=6
                )[:, 0:nx, :]
                nc.scalar.copy(out=outsb[:], in_=psums[(b, ci)][:])
                q = nc.sync if (b * 3 + ci) % 2 == 0 else nc.scalar
                q.dma_start(out=out_d.ap()[:, b, x0 : x0 + nx, :], in_=outsb[:])

            def consume(prod, b, nt):
                """PE: accumulate nt tap-streams of a product tile into psums.
                Chunk-inner order: consecutive matmuls hit different PSUM banks
                (same-bank back-to-back stalls the accumulate pipeline)."""
                for t in range(nt):
                    for ci, (c0, cn) in enumerate(CHUNKS):
                        mm(prod[:, t, c0 : c0 + cn, :], b, ci)
                for ci in range(len(CHUNKS)):
                    evac(b, ci)

            def pair_op(ui, b):
                i, j = UNITS[ui][1]
                prod = ppool.tile(
                    [D, 2, XS, D], f16, name="prod2", tag="prod2", bufs=5
                )
                nc.vector.tensor_mul(
                    out=prod[:], in0=zpair_ap(j, b, i), in1=w_ts[ui][:]
                )
                consume(prod, b, 2)

            def triple_op(ui, b):
                j = UNITS[ui][1]
                prod = ppool.tile(
                    [D, 3, XS, D], f16, name="prod3", tag="prod3", bufs=3
                )
                nc.vector.tensor_mul(
                    out=prod[:], in0=xtriple_ap(j, b), in1=w_ts[ui][:]
                )
                consume(prod, b, 3)

            # -- unit 0: two b0 singles (the first waits only on half of the
            # first W tile). The early ops are b-interleaved: all b0 work for
            # the j=1 z-pairs first, so the X b1 slice is not on the critical
            # path until ~4 us into the streak. --
            i0, j0_ = UNITS[0][1]
            for s, l in enumerate((0, 2)):
                prod = ppool.tile([D, XS, D], f16, name="prod1", tag="prod1", bufs=2)
                nc.vector.tensor_mul(
                    out=prod[:],
                    in0=x_ts[j0_][:, 0, i0 : i0 + XS, l : l + D],
                    in1=w_ts[0][:, s],
                )
                for ci, (c0, cn) in enumerate(CHUNKS):
                    mm(prod[:, c0 : c0 + cn, :], 0, ci)
            pair_op(1, 0)
            pair_op(2, 0)
            prod = ppool.tile([D, 2, XS, D], f16, name="prod2", tag="prod2", bufs=5)
            nc.vector.tensor_mul(out=prod[:], in0=zpair_ap(j0_, 1, i0), in1=w_ts[0][:])
            consume(prod, 1, 2)
            pair_op(1, 1)
            pair_op(2, 1)

            # -- j-grouped middle: each j's l=1 triple right after its pairs --
            for b in range(B):
                triple_op(3, b)
            for ui in (4, 5, 6):
                for b in range(B):
                    pair_op(ui, b)
            for b in range(B):
                triple_op(7, b)
            for ui in (8, 9, 10):
                for b in range(B):
                    pair_op(ui, b)

            # -- last unit (j=2 triple), split per psum chunk so the six
            # accumulations retire staggered --
            j = UNITS[11][1]
            for b in range(B):
                for ci, (x0, nx) in enumerate(CHUNKS):
                    prod = ppool.tile(
                        [D, 3, 5, D], f16, name="prod3c", tag="prod3c", bufs=3
                    )
                    pv = prod[:, :, 0:nx, :] if nx != 5 else prod[:]
                    nc.vector.tensor_mul(
                        out=pv,
                        in0=xtriple_ap(j, b, x0=x0, nx=nx),
                        in1=w_ts[11][:, :, x0 : x0 + nx],
                    )
                    for t in range(3):
                        mm(pv[:, t], b, ci)
                    evac(b, ci)

    nc.compile()
    _GRAPH_CACHE["nc"] = nc
    return nc


def make_in_maps(X, W):
    """Host-side shard prep. X [2,1,96,96,96] f32, W [1,1,96,96,96,27] f32."""
    X = np.asarray(X)
    W = np.asarray(W)
    Xs = X.reshape(B, D, D, D)
    # edge padding on all three spatial dims
    Xp = np.pad(Xs, ((0, 0), (1, 1), (1, 1), (1, 1)), mode="edge")
    # -> [y, b, x, z]
    Xt = np.ascontiguousarray(np.transpose(Xp, (2, 0, 1, 3))).astype(F16)
    W00 = W.reshape(D, D, D, NTAP)  # [x, y, z, tap]
    ident = np.eye(D).astype(F16)
    # [j=0 shift, j=2 shift] lhsT matrices (edge rows doubled to reproduce
    # the replicate padding: pad0 == pad1, pad96 == pad97)
    s0 = np.eye(D, k=1)
    s0[0, 0] = 1.0
    s2 = np.eye(D, k=-1)
    s2[D - 1, D - 1] = 1.0
    shmat = np.ascontiguousarray(np.stack([s0, s2], axis=1)).astype(F16)

    in_maps = []
    for m in range(NCORES):
        xs_full = Xt[:, :, m * XS : m * XS + XH, :]  # [98, 2, 14, 98]
        im = {"ident": ident, "shmat": shmat}
        # only the j=1 copy: x[y, b, x, z] = Xp[y+1, b, x, z]
        im["x"] = np.ascontiguousarray(xs_full[1 : 1 + D])  # [96, 2, 14, 98]
        wm = W00[m * XS : (m + 1) * XS]  # [12, 96, 96, 27]
        wmt = np.transpose(wm, (1, 0, 2, 3))  # [y, x, z, tap]
        blocks = []
        for kind, arg in UNITS:
            taps = _unit_taps(kind, arg)
            idxs = [(i * KSZ + j) * KSZ + l for (i, j, l) in taps]
            blk = wmt[:, :, :, idxs]  # [y, x, z, nt]
            wt = np.transpose(blk, (0, 3, 1, 2))  # [y, nt, x, z]
            blocks.append(wt.reshape(D, -1))
        im["w"] = np.ascontiguousarray(np.concatenate(blocks, axis=1)).astype(F16)
        in_maps.append(im)
    return in_maps


def kernel(X, W):
    global LAST_RESULT
    from concourse.bass_utils import run_bass_kernel_spmd

    nc = _build_graph()
    in_maps = make_in_maps(X, W)
    trace = bool(int(os.environ.get("ASYM_TRACE", "0")))
    res = run_bass_kernel_spmd(
        nc, in_maps, core_ids=list(range(NCORES)), trace=trace
    )
    LAST_RESULT = res

    out = np.empty((B, 1, D, D, D), dtype=np.float32)
    for m in range(NCORES):
        r = res.results[m]["out"].astype(np.float32)  # [y, b, x, z]
        out[:, 0, m * XS : (m + 1) * XS, :, :] = np.transpose(r, (1, 2, 0, 3))
    return out


# revision 12
# speedup vs baseline: 1.2736x; 1.1595x over previous
"""Location-dependent 3D conv (AsymConv) on 8 TRN2 NeuronCores.

Math (per output voxel):
    out[b, 0, x, y, z] = sum_{i,j,l in 0..2} Xp[b, x+i, y+j, z+l] * W[x, y, z, (i*3+j)*3+l]
with Xp = edge-padded X by 1 plane on each spatial side.

Strategy (v6; evolved from the 55.9 us v2 via trace analysis):
  - Shard the X spatial axis (96 = 8 cores x 12 planes); host ships halo'd
    shards -> no inter-core communication.
  - Per-core SBUF layout: partition dim = y (96 of 128); free = (b, x, z).
    y-shifts come from 3 j-copies (j=1 host-shipped; j=0/j=2 built on-chip
    by PE shifted-identity matmuls); x/z shifts are free-dim offsets.
  - Units are consumed J-GROUPED (all j=1 work, then j=0, then j=2, each
    j's l=1 triple right after its z-pairs) so the on-chip j-copy builds
    and z-shift copies get generous deadlines; the early product ops are
    b-interleaved so the X b1 slice is not needed until ~4 us into the
    streak.
  - Products X*W run on the DVE in fp16 2x mode; that stream is the hard
    floor (~34 us busy: 62k free-dim elems/partition at 2 elem/cycle,
    0.96 GHz). Everything else is scheduled to keep the streak stall-free:
      * the two HWDGE rings share ~280 B/ns of HBM bandwidth, so W units
        ALTERNATE between the rings in global consumption order - at any
        moment the rings carry the next two units needed;
      * each dma_start costs ~0.6-0.8 us of ISSUE time on its ring engine,
        so the stream uses few, large transfers;
      * l=1 taps run as i-triples (one DVE op per (j, b) instead of
        pair+single) via an overlapping x-window AP;
      * the last unit (j=2 triple) is split per psum x-chunk so the six
        PSUM accumulations retire staggered; each (b, chunk) is evacuated
        (ACT copy to fp16) and DMA'd out as soon as its 27th tap lands.
  - The 27-term accumulation runs on the TensorEngine as identity matmuls
    into PSUM fp32 (x-chunks 5/5/2 planes), trailing the DVE by <1 op.
"""

import os

import numpy as np

# ---- problem constants (hardcoded per harness rules) ----
B = 2
D = 96  # Dx = Dy = Dz
KSZ = 3
NTAP = KSZ**3  # 27
NCORES = 8
XS = D // NCORES  # 12 x-planes per core
XH = XS + 2  # with halo
ZP = D + 2  # padded z

F16 = np.float16
LAST_RESULT = None  # BassKernelResults of the most recent run (for test.py)

_GRAPH_CACHE = {}

# psum x-plane chunks of the 12-plane streams (480/480/192 columns)
CHUNKS = [(0, 5), (5, 5), (10, 2)]

# ---- W unit schedule (DMA consumption order) ----
# kinds: "z" = (l0, l2) z-pair for (i, j) [2 taps], "t" = i-triple for l=1, j
# [3 taps: (0,j,1), (1,j,1), (2,j,1)].  j=1 units lead: only the j=1 X copy
# is host-shipped; j=0 / j=2 are built on-chip while j=1 computes.
UNITS = [
    ("z", (0, 1)),
    ("z", (1, 1)),
    ("z", (2, 1)),
    ("t", 1),
    ("z", (0, 0)),
    ("z", (1, 0)),
    ("z", (2, 0)),
    ("t", 0),
    ("z", (0, 2)),
    ("z", (1, 2)),
    ("z", (2, 2)),
    ("t", 2),
]


def _unit_taps(kind, arg):
    if kind == "z":
        i, j = arg
        return [(i, j, 0), (i, j, 2)]
    j = arg
    return [(0, j, 1), (1, j, 1), (2, j, 1)]


def _build_graph():
    """Build (and cache) the per-core Bass graph. Same graph for all 8 cores."""
    if "nc" in _GRAPH_CACHE:
        return _GRAPH_CACHE["nc"]

    from concourse import bacc
    from concourse import bass as _bass
    import concourse.mybir as mybir
    from concourse.tile import TileContext

    f16 = mybir.dt.float16
    f32 = mybir.dt.float32

    nc = bacc.Bacc("TRN2", target_bir_lowering=False, debug=False, num_devices=NCORES)

    # only the j=1 X copy is shipped: x[y', b, x, z] = Xp[y'+1, b, x, z];
    # j=0 / j=2 copies are derived on-chip by partition-shift matmuls
    x_d = nc.dram_tensor("x", [D, B, XH, ZP], f16, kind="ExternalInput")
    # W per unit (y-major), all units concatenated along the free dim
    unit_w = [len(_unit_taps(k, a)) * XS * D for (k, a) in UNITS]
    unit_off = np.concatenate([[0], np.cumsum(unit_w)]).tolist()
    w_d = nc.dram_tensor("w", [D, unit_off[-1]], f16, kind="ExternalInput")
    id_d = nc.dram_tensor("ident", [D, D], f16, kind="ExternalInput")
    sh_d = nc.dram_tensor("shmat", [D, 2, D], f16, kind="ExternalInput")
    out_d = nc.dram_tensor("out", [D, B, XS, D], f16, kind="ExternalOutput")

    with TileContext(nc) as tc:
        with (
            tc.tile_pool(name="xp", bufs=1) as xpool,
            tc.tile_pool(name="wp", bufs=1) as wpool,
            tc.tile_pool(name="pp", bufs=6) as ppool,
            tc.tile_pool(name="psp", bufs=1, space="PSUM") as pspool,
        ):
            # ---- static tiles ----
            x_ts = [
                xpool.tile([D, B, XH, ZP], f16, name=f"x_{j}", tag=f"x_{j}")
                for j in range(KSZ)
            ]
            # z-shifted copies for l == 1 (keeps DVE 2x alignment)
            xz_ts = [
                xpool.tile([D, B, XH, ZP], f16, name=f"xz_{j}", tag=f"xz_{j}")
                for j in range(KSZ)
            ]
            w_ts = []
            for ui, (kind, arg) in enumerate(UNITS):
                nt = len(_unit_taps(kind, arg))
                w_ts.append(
                    wpool.tile(
                        [D, nt, XS, D],
                        f16,
                        name=f"w_{ui}",
                        tag="wz" if nt == 2 else "wt",
                        bufs=9 if nt == 2 else 3,
                    )
                )
            id_t = xpool.tile([D, D], f16, name="id_t", tag="id_t")
            sh_t = xpool.tile([D, 2, D], f16, name="sh_t", tag="sh_t")

            # ---- DMA schedule (HWDGE, both rings, consumption order).
            # Few BIG transfers: each dma_start costs ~0.6-0.8 us of issue
            # time on its ring engine, and completion sems lag the data by
            # ~1 us, so fine-grained pieces only starve the stream. ----
            def w_dma(q, ui, half=None):
                o0, o1 = unit_off[ui], unit_off[ui + 1]
                if half is None:
                    q.dma_start(out=w_ts[ui][:], in_=w_d.ap()[:, o0:o1])
                else:
                    h = (o1 - o0) // 2
                    q.dma_start(
                        out=w_ts[ui][:, half : half + 1],
                        in_=w_d.ap()[:, o0 + half * h : o0 + (half + 1) * h],
                    )

            # Units alternate rings in consumption order, and the dma_starts
            # are EMITTED in global consumption order too: the Tile scheduler
            # orders ready instructions by program-order priority and assigns
            # completion-sem lanes round-robin, so emission order is stream
            # order. (Emitting one ring's whole list first scrambles both.)
            nc.scalar.dma_start(out=id_t[:], in_=id_d.ap())
            w_dma(nc.scalar, 0, half=0)
            nc.sync.dma_start(out=x_ts[1][:, 0:1], in_=x_d.ap()[:, 0:1])
            w_dma(nc.scalar, 0, half=1)
            w_dma(nc.sync, 1)
            nc.scalar.dma_start(out=sh_t[:], in_=sh_d.ap())
            nc.sync.dma_start(out=x_ts[1][:, 1:2], in_=x_d.ap()[:, 1:2])
            w_dma(nc.scalar, 2)
            w_dma(nc.scalar, 3)
            w_dma(nc.sync, 4)
            w_dma(nc.scalar, 5)
            w_dma(nc.sync, 6)
            w_dma(nc.scalar, 7)
            w_dma(nc.scalar, 8)
            w_dma(nc.sync, 9)
            w_dma(nc.scalar, 10)
            w_dma(nc.sync, 11)

            # ---- PE: build the j=0 / j=2 X copies from j=1 by partition-
            # shift matmuls; ScalarE evacuates PSUM back to SBUF f16. ACT
            # program order: xz_1 copy first (earliest deadline), then j=0
            # evacs, xz_0, j=2 evacs, xz_2 - each well before its consumer
            # thanks to the j-grouped unit order. ----
            XCH = [(0, 5), (5, 5), (10, 4)]  # XH=14 rows -> <=512 f32 cols
            nc.scalar.copy(
                out=xz_ts[1][:, :, :, 0 : ZP - 1], in_=x_ts[1][:, :, :, 1:ZP]
            )
            for jt, sh_idx in ((0, 0), (2, 1)):
                for b in range(B):
                    for r0, nr in XCH:
                        ps_x = pspool.tile(
                            [D, nr, ZP], f32, name="ps_x", tag="ps_x", bufs=2
                        )
                        nc.tensor.matmul(
                            ps_x[:],
                            sh_t[:, sh_idx, :],
                            x_ts[1][:, b, r0 : r0 + nr, :],
                            start=True,
                            stop=True,
                        )
                        nc.scalar.copy(
                            out=x_ts[jt][:, b, r0 : r0 + nr, :], in_=ps_x[:]
                        )
                nc.scalar.copy(
                    out=xz_ts[jt][:, :, :, 0 : ZP - 1],
                    in_=x_ts[jt][:, :, :, 1:ZP],
                )

            # ---- product + accumulate schedule ----
            psums = {
                (b, ci): pspool.tile(
                    [D, nx, D],
                    f32,
                    name=f"ps_{b}_{ci}",
                    tag="ps5" if nx == 5 else "ps2",
                    bufs=4 if nx == 5 else 2,
                )
                for b in range(B)
                for ci, (x0, nx) in enumerate(CHUNKS)
            }

            def zpair_ap(j, b, i):
                """[D, 2, XS, D] view of x_ts[j]: overlapping z-windows l=0,2."""
                base = x_ts[j][:, b, i : i + XS, 0:D]
                ap = list(base.ap)
                return _bass.AP(
                    base.tensor, base.offset, [ap[0], [2, 2], ap[1], ap[2]]
                )

            def xtriple_ap(j, b, x0=0, nx=XS):
                """[D, 3, nx, D] view of xz_ts[j]: overlapping x-windows
                i=0,1,2 (all l=1 taps of one j in a single op)."""
                base = xz_ts[j][:, b, x0 : x0 + nx, 0:D]
                ap = list(base.ap)
                return _bass.AP(
                    base.tensor, base.offset, [ap[0], [ap[1][0], 3], ap[1], ap[2]]
                )

            # per (b, chunk) accumulation counters for start/stop flags
            seen = {(b, ci): 0 for b in range(B) for ci in range(len(CHUNKS))}
            evacuated = set()

            def mm(prod_slice, b, ci):
                s = seen[(b, ci)]
                nc.tensor.matmul(
                    psums[(b, ci)][:],
                    id_t[:],
                    prod_slice,
                    start=(s == 0),
                    stop=(s == NTAP - 1),
                )
                seen[(b, ci)] = s + 1

            def evac(b, ci):
                """PSUM -> SBUF f16 (ACT) -> DRAM, fired as soon as the 27th
                tap of this (b, chunk) has been accumulated."""
                if (b, ci) in evacuated or seen[(b, ci)] != NTAP:
                    return
                evacuated.add((b, ci))
                x0, nx = CHUNKS[ci]
                outsb = ppool.tile(
                    [D, 5, D], f16, name="outsb", tag="outsb", bufs=6
                )[:, 0:nx, :]
                nc.scalar.copy(out=outsb[:], in_=psums[(b, ci)][:])
                q = nc.sync if (b * 3 + ci) % 2 == 0 else nc.scalar
                q.dma_start(out=out_d.ap()[:, b, x0 : x0 + nx, :], in_=outsb[:])

            def consume(prod, b, nt):
                """PE: accumulate nt tap-streams of a product tile into psums.
                Chunk-inner order: consecutive matmuls hit different PSUM banks
                (same-bank back-to-back stalls the accumulate pipeline)."""
                for t in range(nt):
                    for ci, (c0, cn) in enumerate(CHUNKS):
                        mm(prod[:, t, c0 : c0 + cn, :], b, ci)
                for ci in range(len(CHUNKS)):
                    evac(b, ci)

            def pair_op(ui, b):
                i, j = UNITS[ui][1]
                prod = ppool.tile(
                    [D, 2, XS, D], f16, name="prod2", tag="prod2", bufs=5
                )
                nc.vector.tensor_mul(
                    out=prod[:], in0=zpair_ap(j, b, i), in1=w_ts[ui][:]
                )
                consume(prod, b, 2)

            def triple_op(ui, b):
                j = UNITS[ui][1]
                prod = ppool.tile(
                    [D, 3, XS, D], f16, name="prod3", tag="prod3", bufs=3
                )
                nc.vector.tensor_mul(
                    out=prod[:], in0=xtriple_ap(j, b), in1=w_ts[ui][:]
                )
                consume(prod, b, 3)

            # -- unit 0: two b0 singles (the first waits only on half of the
            # first W tile). The early ops are b-interleaved: all b0 work for
            # the j=1 z-pairs first, so the X b1 slice is not on the critical
            # path until ~4 us into the streak. --
            i0, j0_ = UNITS[0][1]
            for s, l in enumerate((0, 2)):
                prod = ppool.tile([D, XS, D], f16, name="prod1", tag="prod1", bufs=2)
                nc.vector.tensor_mul(
                    out=prod[:],
                    in0=x_ts[j0_][:, 0, i0 : i0 + XS, l : l + D],
                    in1=w_ts[0][:, s],
                )
                for ci, (c0, cn) in enumerate(CHUNKS):
                    mm(prod[:, c0 : c0 + cn, :], 0, ci)
            pair_op(1, 0)
            pair_op(2, 0)
            prod = ppool.tile([D, 2, XS, D], f16, name="prod2", tag="prod2", bufs=5)
            nc.vector.tensor_mul(out=prod[:], in0=zpair_ap(j0_, 1, i0), in1=w_ts[0][:])
            consume(prod, 1, 2)
            pair_op(1, 1)
            pair_op(2, 1)

            # -- j-grouped middle: each j's l=1 triple right after its pairs --
            for b in range(B):
                triple_op(3, b)
            for ui in (4, 5, 6):
                for b in range(B):
                    pair_op(ui, b)
            for b in range(B):
                triple_op(7, b)
            for ui in (8, 9, 10):
                for b in range(B):
                    pair_op(ui, b)

            # -- last unit (j=2 triple), split per psum chunk so the six
            # accumulations retire staggered --
            j = UNITS[11][1]
            for b in range(B):
                for ci, (x0, nx) in enumerate(CHUNKS):
                    prod = ppool.tile(
                        [D, 3, 5, D], f16, name="prod3c", tag="prod3c", bufs=3
                    )
                    pv = prod[:, :, 0:nx, :] if nx != 5 else prod[:]
                    nc.vector.tensor_mul(
                        out=pv,
                        in0=xtriple_ap(j, b, x0=x0, nx=nx),
                        in1=w_ts[11][:, :, x0 : x0 + nx],
                    )
                    for t in range(3):
                        mm(pv[:, t], b, ci)
                    evac(b, ci)

    nc.compile()
    _GRAPH_CACHE["nc"] = nc
    return nc


def make_in_maps(X, W):
    """Host-side shard prep. X [2,1,96,96,96] f32, W [1,1,96,96,96,27] f32."""
    X = np.asarray(X)
    W = np.asarray(W)
    Xs = X.reshape(B, D, D, D)
    # edge padding on all three spatial dims
    Xp = np.pad(Xs, ((0, 0), (1, 1), (1, 1), (1, 1)), mode="edge")
    # -> [y, b, x, z]
    Xt = np.ascontiguousarray(np.transpose(Xp, (2, 0, 1, 3))).astype(F16)
    W00 = W.reshape(D, D, D, NTAP)  # [x, y, z, tap]
    ident = np.eye(D).astype(F16)
    # [j=0 shift, j=2 shift] lhsT matrices (edge rows doubled to reproduce
    # the replicate padding: pad0 == pad1, pad96 == pad97)
    s0 = np.eye(D, k=1)
    s0[0, 0] = 1.0
    s2 = np.eye(D, k=-1)
    s2[D - 1, D - 1] = 1.0
    shmat = np.ascontiguousarray(np.stack([s0, s2], axis=1)).astype(F16)

    in_maps = []
    for m in range(NCORES):
        xs_full = Xt[:, :, m * XS : m * XS + XH, :]  # [98, 2, 14, 98]
        im = {"ident": ident, "shmat": shmat}
        # only the j=1 copy: x[y, b, x, z] = Xp[y+1, b, x, z]
        im["x"] = np.ascontiguousarray(xs_full[1 : 1 + D])  # [96, 2, 14, 98]
        wm = W00[m * XS : (m + 1) * XS]  # [12, 96, 96, 27]
        wmt = np.transpose(wm, (1, 0, 2, 3))  # [y, x, z, tap]
        blocks = []
        for kind, arg in UNITS:
            taps = _unit_taps(kind, arg)
            idxs = [(i * KSZ + j) * KSZ + l for (i, j, l) in taps]
            blk = wmt[:, :, :, idxs]  # [y, x, z, nt]
            wt = np.transpose(blk, (0, 3, 1, 2))  # [y, nt, x, z]
            blocks.append(wt.reshape(D, -1))
        im["w"] = np.ascontiguousarray(np.concatenate(blocks, axis=1)).astype(F16)
        in_maps.append(im)
    return in_maps


def kernel(X, W):
    global LAST_RESULT
    from concourse.bass_utils import run_bass_kernel_spmd

    nc = _build_graph()
    in_maps = make_in_maps(X, W)
    trace = bool(int(os.environ.get("ASYM_TRACE", "0")))
    res = run_bass_kernel_spmd(
        nc, in_maps, core_ids=list(range(NCORES)), trace=trace
    )
    LAST_RESULT = res

    out = np.empty((B, 1, D, D, D), dtype=np.float32)
    for m in range(NCORES):
        r = res.results[m]["out"].astype(np.float32)  # [y, b, x, z]
        out[:, 0, m * XS : (m + 1) * XS, :, :] = np.transpose(r, (1, 2, 0, 3))
    return out


# revision 13
# speedup vs baseline: 1.2911x; 1.0138x over previous
"""Location-dependent 3D conv (AsymConv) on 8 TRN2 NeuronCores.

Math (per output voxel):
    out[b, 0, x, y, z] = sum_{i,j,l in 0..2} Xp[b, x+i, y+j, z+l] * W[x, y, z, (i*3+j)*3+l]
with Xp = edge-padded X by 1 plane on each spatial side.

Strategy (v6; evolved from the 55.9 us v2 via trace analysis):
  - Shard the X spatial axis (96 = 8 cores x 12 planes); host ships halo'd
    shards -> no inter-core communication.
  - Per-core SBUF layout: partition dim = y (96 of 128); free = (b, x, z).
    y-shifts come from 3 j-copies (j=1 host-shipped; j=0/j=2 built on-chip
    by PE shifted-identity matmuls); x/z shifts are free-dim offsets.
  - Units are consumed J-GROUPED (all j=1 work, then j=0, then j=2, each
    j's l=1 triple right after its z-pairs) so the on-chip j-copy builds
    and z-shift copies get generous deadlines; the early product ops are
    b-interleaved so the X b1 slice is not needed until ~4 us into the
    streak.
  - Products X*W run on the DVE in fp16 2x mode; that stream is the hard
    floor (~34 us busy: 62k free-dim elems/partition at 2 elem/cycle,
    0.96 GHz). Everything else is scheduled to keep the streak stall-free:
      * the two HWDGE rings share ~280 B/ns of HBM bandwidth, so W units
        ALTERNATE between the rings in global consumption order - at any
        moment the rings carry the next two units needed;
      * each dma_start costs ~0.6-0.8 us of ISSUE time on its ring engine,
        so the stream uses few, large transfers;
      * l=1 taps run as i-triples (one DVE op per (j, b) instead of
        pair+single) via an overlapping x-window AP;
      * the last unit (j=2 triple) is split per psum x-chunk so the six
        PSUM accumulations retire staggered; each (b, chunk) is evacuated
        (ACT copy to fp16) and DMA'd out as soon as its 27th tap lands.
  - The 27-term accumulation runs on the TensorEngine as identity matmuls
    into PSUM fp32 (x-chunks 5/5/2 planes), trailing the DVE by <1 op.
"""

import os

import numpy as np

# ---- problem constants (hardcoded per harness rules) ----
B = 2
D = 96  # Dx = Dy = Dz
KSZ = 3
NTAP = KSZ**3  # 27
NCORES = 8
XS = D // NCORES  # 12 x-planes per core
XH = XS + 2  # with halo
ZP = D + 2  # padded z

F16 = np.float16
LAST_RESULT = None  # BassKernelResults of the most recent run (for test.py)

_GRAPH_CACHE = {}

# psum x-plane chunks of the 12-plane streams (480/480/192 columns)
CHUNKS = [(0, 5), (5, 5), (10, 2)]

# ---- W unit schedule (DMA consumption order) ----
# kinds: "z" = (l0, l2) z-pair for (i, j) [2 taps], "t" = i-triple for l=1, j
# [3 taps: (0,j,1), (1,j,1), (2,j,1)].  j=1 units lead: only the j=1 X copy
# is host-shipped; j=0 / j=2 are built on-chip while j=1 computes.
UNITS = [
    ("z", (0, 1)),
    ("z", (1, 1)),
    ("z", (2, 1)),
    ("t", 1),
    ("z", (0, 0)),
    ("z", (1, 0)),
    ("z", (2, 0)),
    ("t", 0),
    ("z", (0, 2)),
    ("z", (1, 2)),
    ("z", (2, 2)),
    ("t", 2),
]


def _unit_taps(kind, arg):
    if kind == "z":
        i, j = arg
        return [(i, j, 0), (i, j, 2)]
    j = arg
    return [(0, j, 1), (1, j, 1), (2, j, 1)]


def _build_graph():
    """Build (and cache) the per-core Bass graph. Same graph for all 8 cores."""
    if "nc" in _GRAPH_CACHE:
        return _GRAPH_CACHE["nc"]

    from concourse import bacc
    from concourse import bass as _bass
    import concourse.mybir as mybir
    from concourse.tile import TileContext

    f16 = mybir.dt.float16
    f32 = mybir.dt.float32

    nc = bacc.Bacc("TRN2", target_bir_lowering=False, debug=False, num_devices=NCORES)

    # only the j=1 X copy is shipped: x[y', b, x, z] = Xp[y'+1, b, x, z];
    # j=0 / j=2 copies are derived on-chip by partition-shift matmuls
    x_d = nc.dram_tensor("x", [D, B, XH, ZP], f16, kind="ExternalInput")
    # W per unit (y-major), all units concatenated along the free dim
    unit_w = [len(_unit_taps(k, a)) * XS * D for (k, a) in UNITS]
    unit_off = np.concatenate([[0], np.cumsum(unit_w)]).tolist()
    w_d = nc.dram_tensor("w", [D, unit_off[-1]], f16, kind="ExternalInput")
    id_d = nc.dram_tensor("ident", [D, D], f16, kind="ExternalInput")
    sh_d = nc.dram_tensor("shmat", [D, 2, D], f16, kind="ExternalInput")
    out_d = nc.dram_tensor("out", [D, B, XS, D], f16, kind="ExternalOutput")

    with TileContext(nc) as tc:
        with (
            tc.tile_pool(name="xp", bufs=1) as xpool,
            tc.tile_pool(name="wp", bufs=1) as wpool,
            tc.tile_pool(name="pp", bufs=6) as ppool,
            tc.tile_pool(name="psp", bufs=1, space="PSUM") as pspool,
        ):
            # ---- static tiles ----
            x_ts = [
                xpool.tile([D, B, XH, ZP], f16, name=f"x_{j}", tag=f"x_{j}")
                for j in range(KSZ)
            ]
            # z-shifted copies for l == 1 (keeps DVE 2x alignment)
            xz_ts = [
                xpool.tile([D, B, XH, ZP], f16, name=f"xz_{j}", tag=f"xz_{j}")
                for j in range(KSZ)
            ]
            w_ts = []
            for ui, (kind, arg) in enumerate(UNITS):
                nt = len(_unit_taps(kind, arg))
                w_ts.append(
                    wpool.tile(
                        [D, nt, XS, D],
                        f16,
                        name=f"w_{ui}",
                        tag="wz" if nt == 2 else "wt",
                        bufs=9 if nt == 2 else 3,
                    )
                )
            id_t = xpool.tile([D, D], f16, name="id_t", tag="id_t")
            sh_t = xpool.tile([D, 2, D], f16, name="sh_t", tag="sh_t")

            # ---- DMA schedule (HWDGE, both rings, consumption order).
            # Few BIG transfers: each dma_start costs ~0.6-0.8 us of issue
            # time on its ring engine, and completion sems lag the data by
            # ~1 us, so fine-grained pieces only starve the stream. ----
            def w_dma(q, ui, half=None):
                o0, o1 = unit_off[ui], unit_off[ui + 1]
                if half is None:
                    q.dma_start(out=w_ts[ui][:], in_=w_d.ap()[:, o0:o1])
                else:
                    h = (o1 - o0) // 2
                    q.dma_start(
                        out=w_ts[ui][:, half : half + 1],
                        in_=w_d.ap()[:, o0 + half * h : o0 + (half + 1) * h],
                    )

            # Units alternate rings in consumption order, and the dma_starts
            # are EMITTED in global consumption order too: the Tile scheduler
            # orders ready instructions by program-order priority and assigns
            # completion-sem lanes round-robin, so emission order is stream
            # order. (Emitting one ring's whole list first scrambles both.)
            # The first X/W pieces are split small so the first product op
            # (x-chunk 0 of tap (0,1,0), b0) fires after only ~200 kB.
            o0u = unit_off[0]
            nc.scalar.dma_start(out=id_t[:], in_=id_d.ap())
            nc.scalar.dma_start(
                out=w_ts[0][:, 0:1, 0:5], in_=w_d.ap()[:, o0u : o0u + 5 * D]
            )
            nc.sync.dma_start(out=x_ts[1][:, 0:1, 0:7], in_=x_d.ap()[:, 0:1, 0:7])
            nc.scalar.dma_start(
                out=w_ts[0][:, 0:1, 5:XS],
                in_=w_d.ap()[:, o0u + 5 * D : o0u + XS * D],
            )
            nc.sync.dma_start(out=x_ts[1][:, 0:1, 7:XH], in_=x_d.ap()[:, 0:1, 7:XH])
            w_dma(nc.scalar, 0, half=1)
            w_dma(nc.sync, 1)
            nc.scalar.dma_start(out=sh_t[:], in_=sh_d.ap())
            nc.sync.dma_start(out=x_ts[1][:, 1:2], in_=x_d.ap()[:, 1:2])
            w_dma(nc.scalar, 2)
            w_dma(nc.scalar, 3)
            w_dma(nc.sync, 4)

            # ---- on-chip X prep, interleaved with the LATE W dma issues.
            # A dma issue that blocks on completion-sem recycling stalls all
            # later instructions of its engine, so every late W issue sits
            # BEHIND the ACT work with an earlier deadline.
            # PE builds the j=0 / j=2 X copies from j=1 by partition-shift
            # matmuls; ScalarE evacuates PSUM back to SBUF f16; the z-shifted
            # copies are built per-b so each half unblocks consumers early. ----
            XCH = [(0, 5), (5, 5), (10, 4)]  # XH=14 rows -> <=512 f32 cols

            def xz_copy(j, b):
                nc.scalar.copy(
                    out=xz_ts[j][:, b, :, 0 : ZP - 1], in_=x_ts[j][:, b, :, 1:ZP]
                )

            def build_j(jt, sh_idx):
                for b in range(B):
                    for r0, nr in XCH:
                        ps_x = pspool.tile(
                            [D, nr, ZP], f32, name="ps_x", tag="ps_x", bufs=2
                        )
                        nc.tensor.matmul(
                            ps_x[:],
                            sh_t[:, sh_idx, :],
                            x_ts[1][:, b, r0 : r0 + nr, :],
                            start=True,
                            stop=True,
                        )
                        nc.scalar.copy(
                            out=x_ts[jt][:, b, r0 : r0 + nr, :], in_=ps_x[:]
                        )

            xz_copy(1, 0)
            xz_copy(1, 1)
            w_dma(nc.scalar, 5)
            build_j(0, 0)
            w_dma(nc.sync, 6)
            xz_copy(0, 0)
            xz_copy(0, 1)
            w_dma(nc.scalar, 7)
            build_j(2, 1)
            w_dma(nc.sync, 9)
            xz_copy(2, 0)
            xz_copy(2, 1)
            w_dma(nc.scalar, 8)
            w_dma(nc.scalar, 10)
            w_dma(nc.sync, 11)

            # ---- product + accumulate schedule ----
            psums = {
                (b, ci): pspool.tile(
                    [D, nx, D],
                    f32,
                    name=f"ps_{b}_{ci}",
                    tag="ps5" if nx == 5 else "ps2",
                    bufs=4 if nx == 5 else 2,
                )
                for b in range(B)
                for ci, (x0, nx) in enumerate(CHUNKS)
            }

            def zpair_ap(j, b, i):
                """[D, 2, XS, D] view of x_ts[j]: overlapping z-windows l=0,2."""
                base = x_ts[j][:, b, i : i + XS, 0:D]
                ap = list(base.ap)
                return _bass.AP(
                    base.tensor, base.offset, [ap[0], [2, 2], ap[1], ap[2]]
                )

            def xtriple_ap(j, b, x0=0, nx=XS):
                """[D, 3, nx, D] view of xz_ts[j]: overlapping x-windows
                i=0,1,2 (all l=1 taps of one j in a single op)."""
                base = xz_ts[j][:, b, x0 : x0 + nx, 0:D]
                ap = list(base.ap)
                return _bass.AP(
                    base.tensor, base.offset, [ap[0], [ap[1][0], 3], ap[1], ap[2]]
                )

            # per (b, chunk) accumulation counters for start/stop flags
            seen = {(b, ci): 0 for b in range(B) for ci in range(len(CHUNKS))}
            evacuated = set()

            def mm(prod_slice, b, ci):
                s = seen[(b, ci)]
                nc.tensor.matmul(
                    psums[(b, ci)][:],
                    id_t[:],
                    prod_slice,
                    start=(s == 0),
                    stop=(s == NTAP - 1),
                )
                seen[(b, ci)] = s + 1

            def evac(b, ci):
                """PSUM -> SBUF f16 (ACT) -> DRAM, fired as soon as the 27th
                tap of this (b, chunk) has been accumulated."""
                if (b, ci) in evacuated or seen[(b, ci)] != NTAP:
                    return
                evacuated.add((b, ci))
                x0, nx = CHUNKS[ci]
                outsb = ppool.tile(
                    [D, 5, D], f16, name="outsb", tag="outsb", bufs=6
                )[:, 0:nx, :]
                nc.scalar.copy(out=outsb[:], in_=psums[(b, ci)][:])
                q = nc.sync if (b * 3 + ci) % 2 == 0 else nc.scalar
                q.dma_start(out=out_d.ap()[:, b, x0 : x0 + nx, :], in_=outsb[:])

            def consume(prod, b, nt):
                """PE: accumulate nt tap-streams of a product tile into psums.
                Chunk-inner order: consecutive matmuls hit different PSUM banks
                (same-bank back-to-back stalls the accumulate pipeline)."""
                for t in range(nt):
                    for ci, (c0, cn) in enumerate(CHUNKS):
                        mm(prod[:, t, c0 : c0 + cn, :], b, ci)
                for ci in range(len(CHUNKS)):
                    evac(b, ci)

            def pair_op(ui, b):
                i, j = UNITS[ui][1]
                prod = ppool.tile(
                    [D, 2, XS, D], f16, name="prod2", tag="prod2", bufs=5
                )
                nc.vector.tensor_mul(
                    out=prod[:], in0=zpair_ap(j, b, i), in1=w_ts[ui][:]
                )
                consume(prod, b, 2)

            def triple_op(ui, b):
                j = UNITS[ui][1]
                prod = ppool.tile(
                    [D, 3, XS, D], f16, name="prod3", tag="prod3", bufs=3
                )
                nc.vector.tensor_mul(
                    out=prod[:], in0=xtriple_ap(j, b), in1=w_ts[ui][:]
                )
                consume(prod, b, 3)

            # -- unit 0: two b0 singles (the first waits only on half of the
            # first W tile). The early ops are b-interleaved: all b0 work for
            # the j=1 z-pairs first, so the X b1 slice is not on the critical
            # path until ~4 us into the streak. --
            i0, j0_ = UNITS[0][1]
            prod = ppool.tile([D, 5, D], f16, name="prodc", tag="prodc", bufs=2)
            nc.vector.tensor_mul(
                out=prod[:],
                in0=x_ts[j0_][:, 0, 0:5, 0:D],
                in1=w_ts[0][:, 0, 0:5],
            )
            mm(prod[:], 0, 0)
            prod = ppool.tile([D, 7, D], f16, name="prodd", tag="prodd", bufs=2)
            nc.vector.tensor_mul(
                out=prod[:],
                in0=x_ts[j0_][:, 0, 5:XS, 0:D],
                in1=w_ts[0][:, 0, 5:XS],
            )
            mm(prod[:, 0:5, :], 0, 1)
            mm(prod[:, 5:7, :], 0, 2)
            prod = ppool.tile([D, XS, D], f16, name="prod1", tag="prod1", bufs=2)
            nc.vector.tensor_mul(
                out=prod[:],
                in0=x_ts[j0_][:, 0, i0 : i0 + XS, 2 : 2 + D],
                in1=w_ts[0][:, 1],
            )
            for ci, (c0, cn) in enumerate(CHUNKS):
                mm(prod[:, c0 : c0 + cn, :], 0, ci)
            pair_op(1, 0)
            pair_op(2, 0)
            prod = ppool.tile([D, 2, XS, D], f16, name="prod2", tag="prod2", bufs=5)
            nc.vector.tensor_mul(out=prod[:], in0=zpair_ap(j0_, 1, i0), in1=w_ts[0][:])
            consume(prod, 1, 2)
            pair_op(1, 1)
            pair_op(2, 1)

            # -- j-grouped middle: each j's l=1 triple right after its pairs --
            for b in range(B):
                triple_op(3, b)
            for ui in (4, 5, 6):
                for b in range(B):
                    pair_op(ui, b)
            for b in range(B):
                triple_op(7, b)
            for ui in (8, 9, 10):
                for b in range(B):
                    pair_op(ui, b)

            # -- last unit (j=2 triple), split per psum chunk so the six
            # accumulations retire staggered --
            j = UNITS[11][1]
            for b in range(B):
                for ci, (x0, nx) in enumerate(CHUNKS):
                    prod = ppool.tile(
                        [D, 3, 5, D], f16, name="prod3c", tag="prod3c", bufs=3
                    )
                    pv = prod[:, :, 0:nx, :] if nx != 5 else prod[:]
                    nc.vector.tensor_mul(
                        out=pv,
                        in0=xtriple_ap(j, b, x0=x0, nx=nx),
                        in1=w_ts[11][:, :, x0 : x0 + nx],
                    )
                    for t in range(3):
                        mm(pv[:, t], b, ci)
                    evac(b, ci)

    nc.compile()
    _GRAPH_CACHE["nc"] = nc
    return nc


def make_in_maps(X, W):
    """Host-side shard prep. X [2,1,96,96,96] f32, W [1,1,96,96,96,27] f32."""
    X = np.asarray(X)
    W = np.asarray(W)
    Xs = X.reshape(B, D, D, D)
    # edge padding on all three spatial dims
    Xp = np.pad(Xs, ((0, 0), (1, 1), (1, 1), (1, 1)), mode="edge")
    # -> [y, b, x, z]
    Xt = np.ascontiguousarray(np.transpose(Xp, (2, 0, 1, 3))).astype(F16)
    W00 = W.reshape(D, D, D, NTAP)  # [x, y, z, tap]
    ident = np.eye(D).astype(F16)
    # [j=0 shift, j=2 shift] lhsT matrices (edge rows doubled to reproduce
    # the replicate padding: pad0 == pad1, pad96 == pad97)
    s0 = np.eye(D, k=1)
    s0[0, 0] = 1.0
    s2 = np.eye(D, k=-1)
    s2[D - 1, D - 1] = 1.0
    shmat = np.ascontiguousarray(np.stack([s0, s2], axis=1)).astype(F16)

    in_maps = []
    for m in range(NCORES):
        xs_full = Xt[:, :, m * XS : m * XS + XH, :]  # [98, 2, 14, 98]
        im = {"ident": ident, "shmat": shmat}
        # only the j=1 copy: x[y, b, x, z] = Xp[y+1, b, x, z]
        im["x"] = np.ascontiguousarray(xs_full[1 : 1 + D])  # [96, 2, 14, 98]
        wm = W00[m * XS : (m + 1) * XS]  # [12, 96, 96, 27]
        wmt = np.transpose(wm, (1, 0, 2, 3))  # [y, x, z, tap]
        blocks = []
        for kind, arg in UNITS:
            taps = _unit_taps(kind, arg)
            idxs = [(i * KSZ + j) * KSZ + l for (i, j, l) in taps]
            blk = wmt[:, :, :, idxs]  # [y, x, z, nt]
            wt = np.transpose(blk, (0, 3, 1, 2))  # [y, nt, x, z]
            blocks.append(wt.reshape(D, -1))
        im["w"] = np.ascontiguousarray(np.concatenate(blocks, axis=1)).astype(F16)
        in_maps.append(im)
    return in_maps


def kernel(X, W):
    global LAST_RESULT
    from concourse.bass_utils import run_bass_kernel_spmd

    nc = _build_graph()
    in_maps = make_in_maps(X, W)
    trace = bool(int(os.environ.get("ASYM_TRACE", "0")))
    res = run_bass_kernel_spmd(
        nc, in_maps, core_ids=list(range(NCORES)), trace=trace
    )
    LAST_RESULT = res

    out = np.empty((B, 1, D, D, D), dtype=np.float32)
    for m in range(NCORES):
        r = res.results[m]["out"].astype(np.float32)  # [y, b, x, z]
        out[:, 0, m * XS : (m + 1) * XS, :, :] = np.transpose(r, (1, 2, 0, 3))
    return out
